# revision 21
# baseline (speedup 1.0000x reference)
"""Trainium2 Bass kernel for DeBERTa-style disentangled attention.

Problem: B=16, S=512, D=768, H=12, HD=64, L=512 (att_span), scale=sqrt(3*64).

  Q = q@Wq+bq, K = k@Wk+bk, V = v@Wv+bv   (per-head split)
  scores = (QK^T + c2p + p2c) / scale ; softmax ; ctx = P@V
  c2p[i,j] = Q[i] . pos_k[i-j+512]   (pos_k = rel@Wk+bk, per head)
  p2c[i,j] = K[j] . pos_q[i-j+512]   (pos_q = rel@Wq+bq)
  (clip never binds: i-j+512 in [1,1023])

Sharding: data-parallel over batch, 8 cores x (B_local=2).

Device strategy (per core, everything transposed "scores^T[j,i]"):
  - Projections produce QT/KT [dout, tok] (bf16), V [tok, dout] (bf16,
    augmented with a ones-column per head for softmax denominators),
    PKR = pos_k_reversed^T and PQ = pos_q^T [dout, p] (bf16).
    1/scale folded into Wq/bq on host (covers c2c, c2p via Q; p2c via pos_q).
  - Per (b,h): band matmuls produce c2p_att_rev / p2c_att [128, 640] tiles,
    evacuated bf16 and round-tripped through DRAM; strided re-read
    (row-stride 639, offset 127) yields the diagonal-gathered
    c2p [i,j] / p2cT [j,i] tiles (the DeBERTa "skew" trick).
  - scores^T accumulated in PSUM: c2cT matmul + p2cT via identity-add +
    c2p via PE add-transpose (lhsT=c2p chunk, rhs=identity).
  - exp on ACT (no max subtraction needed: |scores| <~ 3), PV matmul with
    ones-augmented V gives ctx^T and row sums; PE transpose + DVE
    reciprocal/scale finishes ctx = P@V / sums in fp32.
"""

import os
import sys
import numpy as np

for p in ("/opt/trn_rl_repo",):
    if p not in sys.path:
        sys.path.insert(0, p)

import ml_dtypes

import concourse.bass as bass
import concourse.bacc as bacc
import concourse.tile as tile
import concourse.mybir as mybir
from concourse import bass_utils

f32 = mybir.dt.float32
bf16 = mybir.dt.bfloat16
fp8 = mybir.dt.float8e4
FT = mybir.ActivationFunctionType

B, S, D, H = 16, 512, 768, 12
HD = 64
L = 512
P2 = 2 * L  # 1024
NB = 2  # batches per core
NTOK = NB * S  # 1024
NCORES = 8
SCALE = float(np.sqrt(HD * 3.0))
KC = D // 128  # 6 contraction chunks
BW = 640  # band width (pads the 639 used diagonals)
BP = 639  # band row pitch for the skew read

_nbf = ml_dtypes.bfloat16

# ablation / tuning knobs (TimelineSim experiments)
_ABL = set(os.environ.get("KABL", "").split(",")) - {""}
_BUFS = {}
for _kv in os.environ.get("KBUFS", "").split(","):
    if _kv:
        _k, _v = _kv.split("=")
        _BUFS[_k] = int(_v)


def _bufs(name, default):
    return _BUFS.get(name, default)


def build_kernel():
    nc = bacc.Bacc(
        "TRN2",
        target_bir_lowering=False,
        debug=False,
        enable_asserts=False,
        num_devices=NCORES,
    )

    # ---- I/O ----
    qT_d = nc.dram_tensor("qT", [D, NTOK], bf16, kind="ExternalInput")
    kT_d = nc.dram_tensor("kT", [D, NTOK], bf16, kind="ExternalInput")
    vT_d = nc.dram_tensor("vT", [D, NTOK], bf16, kind="ExternalInput")
    wq_d = nc.dram_tensor("Wq", [D, D], bf16, kind="ExternalInput")  # pre-scaled
    wk_d = nc.dram_tensor("Wk", [D, D], bf16, kind="ExternalInput")
    wv_d = nc.dram_tensor("Wv", [D, D], bf16, kind="ExternalInput")
    rT_d = nc.dram_tensor("rT", [D, P2], bf16, kind="ExternalInput")
    rTr_d = nc.dram_tensor("rTrev", [D, P2], bf16, kind="ExternalInput")
    bias_d = nc.dram_tensor("biases", [128, 2 * KC], f32, kind="ExternalInput")
    out_d = nc.dram_tensor("out", [NTOK, D], f32, kind="ExternalOutput")

    idn_np = np.eye(128, dtype=_nbf)
    if "fp8rt" in _ABL:
        idn_np = np.eye(128, dtype=ml_dtypes.float8_e4m3)
    idn_d = nc.inline_tensor(idn_np, name="idn_bf")
    idnf_np = np.eye(128, dtype=np.float32)
    idnf_d = nc.inline_tensor(idnf_np, name="idn_f32")

    with tile.TileContext(nc) as tc:
        _body(nc, tc, qT_d, kT_d, vT_d, wq_d, wk_d, wv_d, rT_d, rTr_d,
              bias_d, idn_d, idnf_d, out_d)
    nc.compile()
    return nc


def _body(nc, tc, qT_d, kT_d, vT_d, wq_d, wk_d, wv_d, rT_d, rTr_d,
          bias_d, idn_d, idnf_d, out_d):
    from contextlib import ExitStack

    with ExitStack() as big:
        const = big.enter_context(tc.tile_pool(name="const", bufs=1))
        acts = big.enter_context(tc.tile_pool(name="acts", bufs=1))

        bdt = fp8 if "fp8rt" in _ABL else bf16
        idn = const.tile([128, 128], bdt)
        nc.sync.dma_start(idn[:], idn_d.ap())
        idnf = const.tile([128, 128], f32)
        nc.sync.dma_start(idnf[:], idnf_d.ap())
        biases = const.tile([128, 2 * KC], f32)
        nc.sync.dma_start(biases[:], bias_d.ap())

        # persistent activations
        QT = [acts.tile([128, NTOK], bf16, name=f"QT{t}") for t in range(KC)]
        KT = [acts.tile([128, NTOK], bf16, name=f"KT{t}") for t in range(KC)]
        PKR = [acts.tile([128, P2 + 1], bf16, name=f"PKR{t}") for t in range(KC)]
        PQ = [acts.tile([128, P2 + 1], bf16, name=f"PQ{t}") for t in range(KC)]
        VA = [acts.tile([128, 65 * H], bf16, name=f"VA{c}") for c in range(8)]

        # ---------------- Stage P: projections ----------------
        with ExitStack() as st:
            inp = st.enter_context(tc.tile_pool(name="inp", bufs=1))
            psp = st.enter_context(
                tc.tile_pool(name="psp", bufs=4, space="PSUM"))

            qT = [inp.tile([128, NTOK], bf16, name=f"qT{t}") for t in range(KC)]
            kTt = [inp.tile([128, NTOK], bf16, name=f"kTt{t}") for t in range(KC)]
            vT = [inp.tile([128, NTOK], bf16, name=f"vT{t}") for t in range(KC)]
            rT = [inp.tile([128, P2], bf16, name=f"rT{t}") for t in range(KC)]
            rTr = [inp.tile([128, P2], bf16, name=f"rTr{t}") for t in range(KC)]
            for t in range(KC):
                sl = slice(128 * t, 128 * (t + 1))
                nc.sync.dma_start(qT[t][:], qT_d.ap()[sl])
                nc.sync.dma_start(kTt[t][:], kT_d.ap()[sl])
                nc.sync.dma_start(vT[t][:], vT_d.ap()[sl])
                nc.sync.dma_start(rT[t][:], rT_d.ap()[sl])
                nc.sync.dma_start(rTr[t][:], rTr_d.ap()[sl])

            wq = [inp.tile([128, D], bf16, name=f"wq{t}") for t in range(KC)]
            wk = [inp.tile([128, D], bf16, name=f"wk{t}") for t in range(KC)]
            wv = [inp.tile([128, D], bf16, name=f"wv{t}") for t in range(KC)]
            for t in range(KC):
                sl = slice(128 * t, 128 * (t + 1))
                nc.sync.dma_start(wq[t][:], wq_d.ap()[sl])
                nc.sync.dma_start(wk[t][:], wk_d.ap()[sl])
                nc.sync.dma_start(wv[t][:], wv_d.ap()[sl])

            # QT / KT / PKR / PQ : out[dout_tile, tok] = W^T @ xT (+ bias)
            for t in range(KC):
                wsl = slice(128 * t, 128 * (t + 1))
                for th in range(2):  # token/pos halves of 512
                    tsl = slice(512 * th, 512 * (th + 1))
                    for (wmat, xin, bcol, dst) in (
                        (wq, qT, 0, QT), (wk, kTt, 1, KT),
                        (wk, rTr, 1, PKR), (wq, rT, 0, PQ),
                    ):
                        ps = psp.tile([128, 512], f32, name="ps_proj",
                                      tag="ps_proj", bufs=4)
                        for kc in range(KC):
                            nc.tensor.matmul(
                                ps[:], wmat[kc][:, wsl], xin[kc][:, tsl],
                                start=(kc == 0), stop=(kc == KC - 1))
                        nc.scalar.activation(
                            dst[t][:, tsl], ps[:], FT.Identity,
                            bias=biases[:, bcol * KC + t : bcol * KC + t + 1],
                            scale=1.0)

            # garbage-pad column P2 of PKR/PQ: zero it
            for t in range(KC):
                nc.vector.memset(PKR[t][:, P2:P2 + 1], 0.0)
                nc.vector.memset(PQ[t][:, P2:P2 + 1], 0.0)

            # V (+ ones cols): out[tok_chunk, dout] = vT^T @ Wv
            for c in range(8):
                csl = slice(128 * c, 128 * (c + 1))
                ps = psp.tile([128, D], f32, name="ps_v", tag="ps_v", bufs=2)
                for osl in (slice(0, 512), slice(512, D)):
                    for kc in range(KC):
                        nc.tensor.matmul(
                            ps[:, osl], vT[kc][:, csl], wv[kc][:, osl],
                            start=(kc == 0), stop=(kc == KC - 1))
                # strided evac: VA[c][:, 65h + d] = ps[:, 64h + d]
                va_v = VA[c][:].rearrange("p (h c) -> p h c", c=65)
                ps_v = ps[:].rearrange("p (h c) -> p h c", c=64)
                nc.vector.tensor_copy(va_v[:, :, 0:64], ps_v)
                nc.vector.memset(va_v[:, :, 64:65], 1.0)

        # ---------------- Stage A: attention ----------------
        with ExitStack() as st:
            dram = st.enter_context(
                tc.tile_pool(name="dramb", bufs=_bufs("dramb", 2), space="DRAM"))
            bsb = st.enter_context(
                tc.tile_pool(name="bsb", bufs=_bufs("bsb", 4)))
            brd = st.enter_context(
                tc.tile_pool(name="brd", bufs=_bufs("brd", 3)))
            expp = st.enter_context(
                tc.tile_pool(name="expp", bufs=_bufs("expp", 2)))
            outp = st.enter_context(tc.tile_pool(name="outp", bufs=2))
            smal = st.enter_context(tc.tile_pool(name="smal", bufs=4))
            ps_band = st.enter_context(
                tc.tile_pool(name="ps_band", bufs=_bufs("ps_band", 2),
                             space="PSUM"))
            ps_sc = st.enter_context(
                tc.tile_pool(name="ps_sc", bufs=_bufs("ps_sc", 2),
                             space="PSUM"))
            ps_ctx = st.enter_context(
                tc.tile_pool(name="ps_ctx", bufs=1, space="PSUM"))
            ps_ctxT = st.enter_context(
                tc.tile_pool(name="ps_ctxT", bufs=1, space="PSUM"))

            for b in range(NB):
                tok0 = 512 * b
                outs = [outp.tile([128, D], f32, name=f"outs{i}",
                                  tag=f"outs{i}") for i in range(4)]
                for hp in range(H // 2):
                    # head pair (2hp, 2hp+1) = rows [0:64] / [64:128] of
                    # SBUF tile hp; K=64 matmuls at base partitions 0/64
                    # are issued adjacently so the PE overlaps them
                    # (row-strip concurrency).
                    th = hp
                    qh = QT[th]
                    kh = KT[th]
                    pkr = PKR[th]
                    pq = PQ[th]
                    RS = (slice(0, 64), slice(64, 128))

                    # --- band matmuls + DRAM roundtrip (both heads) ---
                    cb_d = [dram.tile([4, 128, BW], bdt, name=f"cb_d{s}",
                                      tag=f"cb{s}") for s in range(2)]
                    pb_d = [dram.tile([4, 128, BW], bdt, name=f"pb_d{s}",
                                      tag=f"pb{s}") for s in range(2)]
                    if "nobandmm" not in _ABL:
                        for I in range(4):
                            w0 = 384 - 128 * I
                            pss = [ps_band.tile([128, BW], f32,
                                                name=f"ps_cb{s}",
                                                tag="ps_band")
                                   for s in range(2)]
                            for half in (slice(0, 512), slice(512, BW)):
                                for s in range(2):
                                    lhsT = qh[RS[s], tok0 + 128 * I :
                                              tok0 + 128 * (I + 1)]
                                    nc.tensor.matmul(
                                        pss[s][:, half], lhsT,
                                        pkr[RS[s], w0 + half.start :
                                            w0 + half.stop],
                                        start=True, stop=True)
                            for s in range(2):
                                bb = bsb.tile([128, BW], bdt, name="bb_c",
                                              tag="bb")
                                nc.scalar.activation(bb[:], pss[s][:], FT.Copy)
                                if "nort" not in _ABL:
                                    nc.sync.dma_start(cb_d[s][I], bb[:])
                        for J in range(4):
                            v0 = 385 - 128 * J
                            pss = [ps_band.tile([128, BW], f32,
                                                name=f"ps_pb{s}",
                                                tag="ps_band")
                                   for s in range(2)]
                            for half in (slice(0, 512), slice(512, BW)):
                                for s in range(2):
                                    lhsT = kh[RS[s], tok0 + 128 * J :
                                              tok0 + 128 * (J + 1)]
                                    nc.tensor.matmul(
                                        pss[s][:, half], lhsT,
                                        pq[RS[s], v0 + half.start :
                                           v0 + half.stop],
                                        start=True, stop=True)
                            for s in range(2):
                                bb = bsb.tile([128, BW], bdt, name="bb_p",
                                              tag="bb")
                                nc.vector.tensor_copy(bb[:], pss[s][:])
                                if "nort" not in _ABL:
                                    nc.sync.dma_start(pb_d[s][J], bb[:])

                    # --- skewed (diagonal) re-reads (both heads) ---
                    cbr = [[], []]
                    pbr = [[], []]
                    for s in range(2):
                        for I in range(4):
                            t_ = brd.tile([128, 512], bdt, name=f"cbr{s}{I}",
                                          tag=f"cbr{s}{I}")
                            if "nort" not in _ABL:
                                src = bass.AP(cb_d[s].tensor,
                                              cb_d[s].offset + I * 128 * BW + 127,
                                              [[BP, 128], [1, 512]])
                                nc.sync.dma_start(t_[:], src)
                            cbr[s].append(t_)
                        for J in range(4):
                            t_ = brd.tile([128, 512], bdt, name=f"pbr{s}{J}",
                                          tag=f"pbr{s}{J}")
                            if "nort" not in _ABL:
                                src = bass.AP(pb_d[s].tensor,
                                              pb_d[s].offset + J * 128 * BW + 127,
                                              [[BP, 128], [1, 512]])
                                nc.sync.dma_start(t_[:], src)
                            pbr[s].append(t_)

                    # --- per head: scores^T, exp, PV, ctx ---
                    for s in range(2):
                        h = 2 * hp + s
                        rsl = RS[s]
                        exps = []
                        for J in range(4):
                            ps = ps_sc.tile([128, 512], f32, name="ps_s",
                                            tag="ps_s")
                            noadds = "noadds" in _ABL
                            nc.tensor.matmul(
                                ps[:],
                                kh[rsl, tok0 + 128 * J : tok0 + 128 * (J + 1)],
                                qh[rsl, tok0:tok0 + 512],
                                start=True, stop=noadds)
                            if not noadds:
                                nc.tensor.matmul(ps[:], idn[:], pbr[s][J][:],
                                                 start=False, stop=False)
                                for I in range(4):
                                    nc.tensor.matmul(
                                        ps[:, 128 * I : 128 * (I + 1)],
                                        cbr[s][I][:, 128 * J : 128 * (J + 1)],
                                        idn[:], start=False, stop=(I == 3))
                            e = expp.tile([128, 512], bf16, name=f"exps{J}",
                                          tag=f"exps{J}")
                            nc.scalar.activation(e[:], ps[:], FT.Exp)
                            exps.append(e)

                        # --- PV (ones-augmented) ---
                        pc = ps_ctx.tile([65, 512], f32, name="pc", tag="pc")
                        for J in range(4):
                            nc.tensor.matmul(
                                pc[:], VA[4 * b + J][:, 65 * h : 65 * h + 65],
                                exps[J][:], start=(J == 0), stop=(J == 3))
                        ctxT = smal.tile([65, 512], f32, name="ctxT",
                                         tag="ctxT")
                        nc.vector.tensor_copy(ctxT[:], pc[:])

                        for Ic in range(4):
                            pt = ps_ctxT.tile([128, 65], f32, name="pt",
                                              tag="pt")
                            nc.tensor.transpose(
                                pt[:], ctxT[:, 128 * Ic : 128 * (Ic + 1)],
                                idnf[0:65, 0:65])
                            rec = smal.tile([128, 1], f32, name="rec",
                                            tag="rec")
                            nc.vector.reciprocal(rec[:], pt[:, 64:65])
                            nc.vector.tensor_scalar_mul(
                                outs[Ic][:, 64 * h : 64 * h + 64],
                                pt[:, 0:64], rec[:])

                for Ic in range(4):
                    nc.sync.dma_start(
                        out_d.ap()[tok0 + 128 * Ic : tok0 + 128 * (Ic + 1)],
                        outs[Ic][:])


_NC_CACHE = None
LAST = {}


def _get_nc():
    global _NC_CACHE
    if _NC_CACHE is None:
        _NC_CACHE = build_kernel()
    return _NC_CACHE


def kernel(q, k, v, rel_embeddings, Wq, bq, Wk, bk, Wv, bv, relative_pos,
           **_unused):
    q = np.asarray(q, np.float32)
    k = np.asarray(k, np.float32)
    v = np.asarray(v, np.float32)
    rel = np.asarray(rel_embeddings, np.float32)
    Wq = np.asarray(Wq, np.float32)
    Wk = np.asarray(Wk, np.float32)
    Wv = np.asarray(Wv, np.float32)
    bq = np.asarray(bq, np.float32)
    bk = np.asarray(bk, np.float32)
    bv = np.asarray(bv, np.float32)

    Wq_s, bq_s = Wq / SCALE, bq / SCALE
    wq_b = Wq_s.astype(_nbf)
    wk_b = Wk.astype(_nbf)
    wv_b = Wv.astype(_nbf)
    rT = np.ascontiguousarray(rel.T).astype(_nbf)
    rTr = np.ascontiguousarray(rel[::-1].T).astype(_nbf)
    biases = np.stack([bq_s.reshape(KC, 128), bk.reshape(KC, 128)], 0)
    biases = np.ascontiguousarray(
        biases.reshape(2 * KC, 128).T).astype(np.float32)  # [128, 2*KC]

    in_maps = []
    for c in range(NCORES):
        bs = [NB * c + i for i in range(NB)]
        qT = np.ascontiguousarray(
            np.concatenate([q[b].T for b in bs], axis=1)).astype(_nbf)
        kT = np.ascontiguousarray(
            np.concatenate([k[b].T for b in bs], axis=1)).astype(_nbf)
        vT = np.ascontiguousarray(
            np.concatenate([v[b].T for b in bs], axis=1)).astype(_nbf)
        in_maps.append({
            "qT": qT, "kT": kT, "vT": vT,
            "Wq": wq_b, "Wk": wk_b, "Wv": wv_b,
            "rT": rT, "rTrev": rTr, "biases": biases,
        })

    nc = _get_nc()
    res = bass_utils.run_bass_kernel_spmd(
        nc, in_maps, core_ids=list(range(NCORES)),
        trace=bool(int(os.environ.get("KTRACE", "0"))))
    LAST["res"] = res
    out = np.empty((B, S, D), np.float32)
    for c in range(NCORES):
        o = res.results[c]["out"].reshape(NB, S, D)
        for i in range(NB):
            out[NB * c + i] = o[i]
    return out


if __name__ == "__main__":
    nc = build_kernel()
    print("built ok")
